# revision 16
# baseline (speedup 1.0000x reference)
"""Single-head causal attention (B=8, T=2048, C=768, H=64) on 8 TRN2 NeuronCores.

Sharding: data-parallel over the batch dim - one batch element per core.

Per-core algorithm (bf16 matmul operands, fp32 PSUM accumulation):
  - xT [C, T] bf16 fed from host; DMA'd in four 512-column stripes, each split
    across both HWDGE queues (scalar: chunks 0:3, sync: chunks 3:6) so the
    first QKV matmuls start as early as possible. No warmup matmuls: the
    projection work itself warms the PE while the DMA is still streaming.
  - qkT [128, T]: rows 0:64 = q^T, 64:128 = k^T (fused [Wq | Wk] weights,
    xT chunk moving); k^T half shifted to a base-0 tile via SBUF->SBUF DMA.
  - v computed directly in NATURAL layout (no transposes): per (t-chunk, c):
    lhsT = xT chunk [128c, 128t] (stationary, FWL), rhs = Wv [128c, 64] moving
    -> psum [128t, 64]. Half the moving columns of the streaming formulation.
  - attention in S^T layout (keys j on partitions, queries i on free):
    S^T(j-chunk, i-range) = kT_j.T @ qT, 1024-wide column groups. Scale+exp
    fused on ScalarE (PSUM -> SBUF bf16). Causal: only j <= i blocks are
    computed; leading 128-col diagonal block gets an upper-tri mask multiply.
  - AV: out^T [65, group] += [v_j | 1].T @ expS^T_j; row 64 accumulates the
    softmax denominators for free.
  - QKV projection stripes 2,3 are interleaved into the group-0 attention
    pipeline so the PE, ScalarE (exp), and DVE all stay busy concurrently.
  - finalize per 512-col half as soon as its last AV lands: copy out^T half to
    SBUF, PE-transpose [65,128] blocks to [128,65] fp32, per-partition
    reciprocal of col 64, tensor_scalar multiply -> natural bf16 output
    [128, 4, 64], ONE merged output DMA per half (4 total).

No max-subtraction in softmax: scores * C**-0.5 are bounded (|s| < ~3), exp is
safe in fp32, and the result is mathematically identical to jax.nn.softmax.
Output is bf16 on-device (rel err ~4e-3 total), upcast to f32 on the host.
"""

import ml_dtypes
import numpy as np

import concourse.bass as bass
import concourse.tile as tile
from concourse import bacc, mybir
from concourse.bass import ds, ts
from concourse.masks import make_identity, make_upper_triangular

B, T, C, H = 8, 2048, 768, 64
P = 128
NCH = C // P          # 6 contraction chunks for QKV
GW = 1024             # attention output column-group width
NG = T // GW          # 2 groups
NT = T // P           # 16 t-chunks
JPG = GW // P         # 8 j-chunks per group
SCALE = float(C) ** -0.5

F32 = mybir.dt.float32
BF16 = mybir.dt.bfloat16
EXP = mybir.ActivationFunctionType.Exp


def _emit(tc: tile.TileContext, ctx, xT, wqk, wv, out):
    nc = tc.nc

    consts = ctx.enter_context(tc.tile_pool(name="consts", bufs=1))
    xpool = ctx.enter_context(tc.tile_pool(name="x", bufs=1))
    qpool = ctx.enter_context(tc.tile_pool(name="qkv", bufs=1))

    ident65 = consts.tile([H + 1, H + 1], BF16)
    make_identity(nc, ident65[:])
    tri = consts.tile([P, P], BF16)
    make_upper_triangular(nc, tri[:], val=1.0, diag=True)

    # weights split across the two HWDGE queues to balance the head load
    w_qk = consts.tile([P, NCH, P], BF16)
    nc.scalar.dma_start(w_qk[:], wqk.rearrange("(o p) m -> p o m", p=P))
    w_v = consts.tile([P, NCH, H], BF16)
    nc.sync.dma_start(w_v[:], wv.rearrange("(o p) m -> p o m", p=P))

    # xT in four 512-col stripes, each split across the two HWDGE queues.
    # Only stripes 0-1 of the sync half go out up front: the sync queue is
    # FIFO, and the kT shifts (which gate all S^T matmuls) must not sit
    # behind the whole x stream. x2B/x3B are emitted between the kT shifts.
    xT_sb = xpool.tile([P, NCH, T], BF16)
    xTr = xT.rearrange("(o p) t -> p o t", p=P)
    for s in range(4):
        nc.scalar.dma_start(xT_sb[:, 0:3, ts(s, 512)], xTr[:, 0:3, ts(s, 512)])
        nc.sync.dma_start(xT_sb[:, 3:6, ts(s, 512)], xTr[:, 3:6, ts(s, 512)])

    qkT = qpool.tile([P, T], BF16)
    kT = qpool.tile([H, T], BF16)
    v_sb = qpool.tile([P, NT, H + 1], BF16)
    nc.vector.memset(v_sb[:, :, H : H + 1], 1.0)

    # warmup: dummy matmuls fill the PE while the first x stripes stream in,
    # so the HAM clock gate is already released when real work arrives
    dum = qpool.tile([P, 512], BF16)
    nc.vector.memset(dum[:], 0.0)

    with tc.tile_pool(name="warm", bufs=2, space="PSUM") as wp:
        for w in range(12):
            dps = wp.tile([P, 512], F32, tag="w", name=f"warm_{w}")
            nc.tensor.matmul(dps[:], dum[:, 0:P], dum[:], start=True, stop=True)

    # shared PSUM pools: sp serves both projection stripes and S^T chunks
    sp = ctx.enter_context(tc.tile_pool(name="spsum", bufs=2, space="PSUM"))
    op = ctx.enter_context(tc.tile_pool(name="opsum", bufs=1, space="PSUM"))
    fp = ctx.enter_context(tc.tile_pool(name="fpsum", bufs=2, space="PSUM"))
    pb = ctx.enter_context(tc.tile_pool(name="probs", bufs=6))
    fin = ctx.enter_context(tc.tile_pool(name="fin", bufs=3))

    def emit_proj_qk(s):
        # one 512-col stripe of q|k transposed (fused [Wq | Wk] weights)
        ps = sp.tile([P, GW], F32, tag="s", name=f"projqk_{s}")
        for c in range(NCH):
            nc.tensor.matmul(
                ps[:, 0:512],
                w_qk[:, c, :],
                xT_sb[:, c, ts(s, 512)],
                start=(c == 0),
                stop=(c == NCH - 1),
            )
        nc.vector.tensor_copy(qkT[:, ts(s, 512)], ps[:, 0:512])
        # k^T shift to base partition 0 on the gpsimd software-DGE queue:
        # keeps it off the FIFO hardware queues that carry the bulk x stream
        nc.gpsimd.dma_start(kT[:, ts(s, 512)], qkT[H:P, ts(s, 512)])

    def emit_proj_v(s):
        # v in natural layout: xT chunk stationary (FWL), Wv moving
        ps = sp.tile([P, GW], F32, tag="s", name=f"projv_{s}")
        for i in range(4):
            t = 4 * s + i
            reg = ps[:, ds(H * i, H)]
            for c in range(NCH):
                nc.tensor.matmul(
                    reg,
                    xT_sb[:, c, ts(t, P)],
                    w_v[:, c, :],
                    start=(c == 0),
                    stop=(c == NCH - 1),
                )
            nc.vector.tensor_copy(v_sb[:, t, 0:H], reg)

    def emit_probs(g, jj):
        istart = max(g * GW, jj * P)
        n = (g + 1) * GW - istart
        sps = sp.tile([P, GW], F32, tag="s")
        for h in range(0, n, 512):
            nh = min(512, n - h)
            nc.tensor.matmul(
                sps[:, h : h + nh],
                kT[:, ts(jj, P)],
                qkT[0:H, ds(istart + h, nh)],
                start=True,
                stop=True,
            )
        prb = pb.tile([P, GW], BF16, tag="p")
        nc.scalar.activation(prb[:, :n], sps[:, :n], EXP, scale=SCALE)
        if jj >= JPG * g:
            # leading 128 cols are the diagonal block: upper-tri (j<=i) mask
            nc.vector.tensor_mul(out=prb[:, :P], in0=prb[:, :P], in1=tri[:])
        return prb

    def emit_finalize(g, hh, ops):
        ot = fin.tile([H + 1, 512], BF16, tag="ot", name=f"ot_{g}_{hh}")
        nc.vector.tensor_copy(ot[:], ops[:, ts(hh, 512)])
        o_nat = fin.tile([P, 4, H], BF16, tag="onat", name=f"onat_{g}_{hh}")
        for t8 in range(4):
            ftile = fp.tile([P, H + 1], BF16, tag="ft", name=f"ft_{g}_{hh}_{t8}")
            nc.tensor.transpose(ftile[:], ot[:, ts(t8, P)], ident65[:])
            rch = fin.tile([P, 1], F32, tag="rch")
            nc.vector.reciprocal(rch[:], ftile[:, H : H + 1])
            nc.vector.tensor_scalar_mul(o_nat[:, t8, :], ftile[:, 0:H], rch[:])
        ov = out.rearrange("(a p) h -> p a h", p=P)
        nc.sync.dma_start(ov[:, ds(g * 8 + hh * 4, 4), :], o_nat[:])

    emit_proj_qk(0)
    emit_proj_v(0)
    emit_proj_qk(1)
    emit_proj_v(1)

    # projection stripes 2/3 interleave into group-0 attention; they must be
    # emitted on the PE queue before anything that consumes their outputs, but
    # late enough that stripe-{2,3} DMA data has landed by the time the PE
    # reaches them (avoids head-of-line blocking + a HAM warmup reset).
    inject = {3: [("qk", 2)], 4: [("v", 2)], 5: [("qk", 3)], 6: [("v", 3)]}

    ops_by_g = {}
    pending_finalize = None
    LOOKAHEAD = 2
    prb_queue = []
    idx = 0
    for g in range(NG):
        npair = JPG * g + JPG
        prb_queue += [emit_probs(g, jj) for jj in range(LOOKAHEAD)]
        for jj in range(npair):
            prb = prb_queue.pop(0)
            if jj + LOOKAHEAD < npair:
                prb_queue.append(emit_probs(g, jj + LOOKAHEAD))

            if jj == 0:
                ops_by_g[g] = op.tile([H + 1, GW], F32, tag="o", name=f"ops_{g}")
            ops = ops_by_g[g]
            istart = max(g * GW, jj * P)
            n = (g + 1) * GW - istart
            ioff = istart - g * GW
            # split at the ops tile's absolute 512-col PSUM bank boundaries
            seg = ioff
            while seg < ioff + n:
                seg_end = min(ioff + n, (seg // 512 + 1) * 512)
                half = seg // 512
                # last j-chunk writing this 512-wide half of the group
                jj_last = min(JPG * g + JPG - 1, JPG * g + 4 * (half + 1) - 1)
                nc.tensor.matmul(
                    ops[:, seg:seg_end],
                    v_sb[:, jj, :],
                    prb[:, seg - ioff : seg_end - ioff],
                    start=(jj == 0),
                    stop=(jj == jj_last),
                )
                seg = seg_end

            for kind, s in inject.get(idx, ()):
                if kind == "qk":
                    emit_proj_qk(s)
                else:
                    emit_proj_v(s)

            # delayed by one pair so the finalize copy overlaps the next
            # pair's matmuls instead of stalling the PE
            if pending_finalize is not None:
                emit_finalize(*pending_finalize)
                pending_finalize = None
            if jj == JPG * g + 3:
                pending_finalize = (g, 0, ops)
            elif jj == JPG * g + JPG - 1:
                pending_finalize = (g, 1, ops)
            idx += 1
    emit_finalize(*pending_finalize)


def build():
    from contextlib import ExitStack

    nc = bacc.Bacc("TRN2", target_bir_lowering=False, debug=False, num_devices=B)
    xT = nc.dram_tensor("xT", [C, T], BF16, kind="ExternalInput").ap()
    wqk = nc.dram_tensor("wqk", [C, P], BF16, kind="ExternalInput").ap()
    wv = nc.dram_tensor("wv", [C, H], BF16, kind="ExternalInput").ap()
    out = nc.dram_tensor("o", [T, H], BF16, kind="ExternalOutput").ap()
    with tile.TileContext(nc) as tc, ExitStack() as ctx:
        _emit(tc, ctx, xT, wqk, wv, out)
    nc.compile()
    return nc


_NC = None


def _get_nc():
    global _NC
    if _NC is None:
        _NC = build()
    return _NC


def make_in_maps(x, Wk, Wq, Wv):
    bf = ml_dtypes.bfloat16
    wqk = np.ascontiguousarray(np.concatenate([Wq, Wk], axis=1)).astype(bf)
    wv = np.ascontiguousarray(np.asarray(Wv)).astype(bf)
    return [
        {
            "xT": np.ascontiguousarray(np.asarray(x[b]).T).astype(bf),
            "wqk": wqk,
            "wv": wv,
        }
        for b in range(B)
    ]


def kernel(x, Wk, Wq, Wv):
    from concourse.bass_utils import run_bass_kernel_spmd

    nc = _get_nc()
    in_maps = make_in_maps(x, Wk, Wq, Wv)
    r = run_bass_kernel_spmd(nc, in_maps, core_ids=list(range(B)))
    out = np.stack([np.asarray(r.results[b]["o"]) for b in range(B)])
    return np.ascontiguousarray(out.astype(np.float32))


# revision 19
# speedup vs baseline: 1.1218x; 1.1218x over previous
"""Single-head causal attention (B=8, T=2048, C=768, H=64) on 8 TRN2 NeuronCores.

Sharding: data-parallel over the batch dim - one batch element per core.

Per-core algorithm (bf16 matmul operands, fp32 PSUM accumulation):
  - xT [C, T] bf16 fed from host; DMA'd in four 512-column stripes, each split
    across both HWDGE queues (scalar: chunks 0:3, sync: chunks 3:6) so the
    first QKV matmuls start as early as possible. No warmup matmuls: the
    projection work itself warms the PE while the DMA is still streaming.
  - qkT [128, T]: rows 0:64 = q^T, 64:128 = k^T (fused [Wq | Wk] weights,
    xT chunk moving); k^T half shifted to a base-0 tile via SBUF->SBUF DMA.
  - v computed directly in NATURAL layout (no transposes): per (t-chunk, c):
    lhsT = xT chunk [128c, 128t] (stationary, FWL), rhs = Wv [128c, 64] moving
    -> psum [128t, 64]. Half the moving columns of the streaming formulation.
  - attention in S^T layout (keys j on partitions, queries i on free):
    S^T(j-chunk, i-range) = kT_j.T @ qT, 1024-wide column groups. Scale+exp
    fused on ScalarE (PSUM -> SBUF bf16). Causal: only j <= i blocks are
    computed; leading 128-col diagonal block gets an upper-tri mask multiply.
  - AV: out^T [65, group] += [v_j | 1].T @ expS^T_j; row 64 accumulates the
    softmax denominators for free.
  - QKV projection stripes 2,3 are interleaved into the group-0 attention
    pipeline so the PE, ScalarE (exp), and DVE all stay busy concurrently.
  - finalize per 512-col half as soon as its last AV lands: copy out^T half to
    SBUF, PE-transpose [65,128] blocks to [128,65] fp32, per-partition
    reciprocal of col 64, tensor_scalar multiply -> natural bf16 output
    [128, 4, 64], ONE merged output DMA per half (4 total).

No max-subtraction in softmax: scores * C**-0.5 are bounded (|s| < ~3), exp is
safe in fp32, and the result is mathematically identical to jax.nn.softmax.
Output is bf16 on-device (rel err ~4e-3 total), upcast to f32 on the host.
"""

import ml_dtypes
import numpy as np

import concourse.bass as bass
import concourse.tile as tile
from concourse import bacc, mybir
from concourse.bass import ds, ts
from concourse.masks import make_identity, make_upper_triangular

B, T, C, H = 8, 2048, 768, 64
P = 128
NCH = C // P          # 6 contraction chunks for QKV
GW = 1024             # attention output column-group width
NG = T // GW          # 2 groups
NT = T // P           # 16 t-chunks
JPG = GW // P         # 8 j-chunks per group
SCALE = float(C) ** -0.5

F32 = mybir.dt.float32
BF16 = mybir.dt.bfloat16
EXP = mybir.ActivationFunctionType.Exp


def _emit(tc: tile.TileContext, ctx, xT, wqk, wv, out):
    nc = tc.nc

    consts = ctx.enter_context(tc.tile_pool(name="consts", bufs=1))
    xpool = ctx.enter_context(tc.tile_pool(name="x", bufs=1))
    qpool = ctx.enter_context(tc.tile_pool(name="qkv", bufs=1))

    ident65 = consts.tile([H + 1, H + 1], BF16)
    make_identity(nc, ident65[:])
    tri = consts.tile([P, P], BF16)
    make_upper_triangular(nc, tri[:], val=1.0, diag=True)

    # weights split across the two HWDGE queues to balance the head load
    w_qk = consts.tile([P, NCH, P], BF16)
    nc.scalar.dma_start(w_qk[:], wqk.rearrange("(o p) m -> p o m", p=P))
    w_v = consts.tile([P, NCH, H], BF16)
    nc.sync.dma_start(w_v[:], wv.rearrange("(o p) m -> p o m", p=P))

    # xT in four 512-col stripes, each split across the two HWDGE queues.
    # Only stripes 0-1 of the sync half go out up front: the sync queue is
    # FIFO, and the kT shifts (which gate all S^T matmuls) must not sit
    # behind the whole x stream. x2B/x3B are emitted between the kT shifts.
    xT_sb = xpool.tile([P, NCH, T], BF16)
    xTr = xT.rearrange("(o p) t -> p o t", p=P)
    for s in range(4):
        nc.scalar.dma_start(xT_sb[:, 0:3, ts(s, 512)], xTr[:, 0:3, ts(s, 512)])
        nc.sync.dma_start(xT_sb[:, 3:6, ts(s, 512)], xTr[:, 3:6, ts(s, 512)])

    qkT = qpool.tile([P, T], BF16)
    kT = qpool.tile([H, T], BF16)
    v_sb = qpool.tile([P, NT, H + 1], BF16)
    nc.vector.memset(v_sb[:, :, H : H + 1], 1.0)

    # warmup: dummy matmuls fill the PE while the first x stripes stream in,
    # so the HAM clock gate is already released when real work arrives
    dum = qpool.tile([P, 512], BF16)
    nc.vector.memset(dum[:], 0.0)

    with tc.tile_pool(name="warm", bufs=2, space="PSUM") as wp:
        for w in range(12):
            dps = wp.tile([P, 512], F32, tag="w", name=f"warm_{w}")
            nc.tensor.matmul(dps[:], dum[:, 0:P], dum[:], start=True, stop=True)

    # shared PSUM pools: sp serves both projection stripes and S^T chunks
    sp = ctx.enter_context(tc.tile_pool(name="spsum", bufs=2, space="PSUM"))
    op = ctx.enter_context(tc.tile_pool(name="opsum", bufs=1, space="PSUM"))
    fp = ctx.enter_context(tc.tile_pool(name="fpsum", bufs=2, space="PSUM"))
    pb = ctx.enter_context(tc.tile_pool(name="probs", bufs=6))
    fin = ctx.enter_context(tc.tile_pool(name="fin", bufs=3))

    def emit_proj_qk(s):
        # one 512-col stripe of q|k transposed (fused [Wq | Wk] weights)
        ps = sp.tile([P, GW], F32, tag="s", name=f"projqk_{s}")
        for c in range(NCH):
            nc.tensor.matmul(
                ps[:, 0:512],
                w_qk[:, c, :],
                xT_sb[:, c, ts(s, 512)],
                start=(c == 0),
                stop=(c == NCH - 1),
            )
        nc.vector.tensor_copy(qkT[:, ts(s, 512)], ps[:, 0:512])
        # k^T shift to base partition 0 on the gpsimd software-DGE queue:
        # keeps it off the FIFO hardware queues that carry the bulk x stream
        nc.gpsimd.dma_start(kT[:, ts(s, 512)], qkT[H:P, ts(s, 512)])

    def emit_proj_v(s):
        # v in natural layout: xT chunk stationary (FWL), Wv moving
        ps = sp.tile([P, GW], F32, tag="s", name=f"projv_{s}")
        for i in range(4):
            t = 4 * s + i
            reg = ps[:, ds(H * i, H)]
            for c in range(NCH):
                nc.tensor.matmul(
                    reg,
                    xT_sb[:, c, ts(t, P)],
                    w_v[:, c, :],
                    start=(c == 0),
                    stop=(c == NCH - 1),
                )
            nc.vector.tensor_copy(v_sb[:, t, 0:H], reg)

    def emit_probs(g, jj):
        istart = max(g * GW, jj * P)
        n = (g + 1) * GW - istart
        sps = sp.tile([P, GW], F32, tag="s")
        for h in range(0, n, 512):
            nh = min(512, n - h)
            nc.tensor.matmul(
                sps[:, h : h + nh],
                kT[:, ts(jj, P)],
                qkT[0:H, ds(istart + h, nh)],
                start=True,
                stop=True,
            )
        prb = pb.tile([P, GW], BF16, tag="p")
        nc.scalar.activation(prb[:, :n], sps[:, :n], EXP, scale=SCALE)
        if jj >= JPG * g:
            # leading 128 cols are the diagonal block: upper-tri (j<=i) mask
            nc.vector.tensor_mul(out=prb[:, :P], in0=prb[:, :P], in1=tri[:])
        return prb

    def emit_finalize(g, hh, ops):
        ot = fin.tile([H + 1, 512], BF16, tag="ot", name=f"ot_{g}_{hh}")
        if g == NG - 1:
            # ScalarE is done with exp by now; offload the copy from the
            # congested vector queue to shorten the tail
            nc.scalar.copy(ot[:], ops[:, ts(hh, 512)])
        else:
            nc.vector.tensor_copy(ot[:], ops[:, ts(hh, 512)])
        o_nat = fin.tile([P, 4, H], BF16, tag="onat", name=f"onat_{g}_{hh}")
        for t8 in range(4):
            ftile = fp.tile([P, H + 1], BF16, tag="ft", name=f"ft_{g}_{hh}_{t8}")
            nc.tensor.transpose(ftile[:], ot[:, ts(t8, P)], ident65[:])
            rch = fin.tile([P, 1], F32, tag="rch")
            nc.vector.reciprocal(rch[:], ftile[:, H : H + 1])
            nc.vector.tensor_scalar_mul(o_nat[:, t8, :], ftile[:, 0:H], rch[:])
        ov = out.rearrange("(a p) h -> p a h", p=P)
        nc.sync.dma_start(ov[:, ds(g * 8 + hh * 4, 4), :], o_nat[:])

    # QK stripes 0-1 first: the S^T chain (and thus ScalarE's 21us exp
    # stream) depends only on these, not on the v projections.
    emit_proj_qk(0)
    emit_proj_qk(1)

    # remaining projection units interleave into group-0 attention; they must
    # be emitted on the PE queue before anything that consumes their outputs,
    # but late enough that their input data has landed by the time the PE
    # reaches them (avoids head-of-line blocking + a HAM warmup reset).
    inject = {
        0: [("v", 1)],
        3: [("qk", 2)],
        4: [("v", 2)],
        5: [("qk", 3)],
        6: [("v", 3)],
    }

    ops_by_g = {}
    pending_finalize = None
    LOOKAHEAD = 2
    prb_queue = []
    idx = 0
    for g in range(NG):
        npair = JPG * g + JPG
        prb_queue += [emit_probs(g, jj) for jj in range(LOOKAHEAD)]
        if g == 0:
            emit_proj_v(0)
        for jj in range(npair):
            prb = prb_queue.pop(0)
            if jj + LOOKAHEAD < npair:
                prb_queue.append(emit_probs(g, jj + LOOKAHEAD))

            if jj == 0:
                ops_by_g[g] = op.tile([H + 1, GW], F32, tag="o", name=f"ops_{g}")
            ops = ops_by_g[g]
            istart = max(g * GW, jj * P)
            n = (g + 1) * GW - istart
            ioff = istart - g * GW
            # split at the ops tile's absolute 512-col PSUM bank boundaries
            seg = ioff
            while seg < ioff + n:
                seg_end = min(ioff + n, (seg // 512 + 1) * 512)
                half = seg // 512
                # last j-chunk writing this 512-wide half of the group
                jj_last = min(JPG * g + JPG - 1, JPG * g + 4 * (half + 1) - 1)
                nc.tensor.matmul(
                    ops[:, seg:seg_end],
                    v_sb[:, jj, :],
                    prb[:, seg - ioff : seg_end - ioff],
                    start=(jj == 0),
                    stop=(jj == jj_last),
                )
                seg = seg_end

            for kind, s in inject.get(idx, ()):
                if kind == "qk":
                    emit_proj_qk(s)
                else:
                    emit_proj_v(s)

            # delayed by one pair so the finalize copy overlaps the next
            # pair's matmuls instead of stalling the PE
            if pending_finalize is not None:
                emit_finalize(*pending_finalize)
                pending_finalize = None
            if jj == JPG * g + 3:
                pending_finalize = (g, 0, ops)
            elif jj == JPG * g + JPG - 1:
                pending_finalize = (g, 1, ops)
            idx += 1
    emit_finalize(*pending_finalize)


def build():
    from contextlib import ExitStack

    nc = bacc.Bacc("TRN2", target_bir_lowering=False, debug=False, num_devices=B)
    xT = nc.dram_tensor("xT", [C, T], BF16, kind="ExternalInput").ap()
    wqk = nc.dram_tensor("wqk", [C, P], BF16, kind="ExternalInput").ap()
    wv = nc.dram_tensor("wv", [C, H], BF16, kind="ExternalInput").ap()
    out = nc.dram_tensor("o", [T, H], BF16, kind="ExternalOutput").ap()
    with tile.TileContext(nc) as tc, ExitStack() as ctx:
        _emit(tc, ctx, xT, wqk, wv, out)
    nc.compile()
    return nc


_NC = None


def _get_nc():
    global _NC
    if _NC is None:
        _NC = build()
    return _NC


def make_in_maps(x, Wk, Wq, Wv):
    bf = ml_dtypes.bfloat16
    wqk = np.ascontiguousarray(np.concatenate([Wq, Wk], axis=1)).astype(bf)
    wv = np.ascontiguousarray(np.asarray(Wv)).astype(bf)
    return [
        {
            "xT": np.ascontiguousarray(np.asarray(x[b]).T).astype(bf),
            "wqk": wqk,
            "wv": wv,
        }
        for b in range(B)
    ]


def kernel(x, Wk, Wq, Wv):
    from concourse.bass_utils import run_bass_kernel_spmd

    nc = _get_nc()
    in_maps = make_in_maps(x, Wk, Wq, Wv)
    r = run_bass_kernel_spmd(nc, in_maps, core_ids=list(range(B)))
    out = np.stack([np.asarray(r.results[b]["o"]) for b in range(B)])
    return np.ascontiguousarray(out.astype(np.float32))


# revision 24
# speedup vs baseline: 1.2514x; 1.1155x over previous
"""Single-head causal attention (B=8, T=2048, C=768, H=64) on 8 TRN2 NeuronCores.

Sharding: data-parallel over the batch dim - one batch element per core.

Per-core algorithm (bf16 matmul operands, fp32 PSUM accumulation):
  - xT [C, T] bf16 fed from host; DMA'd in four 512-column stripes, each split
    across both HWDGE queues. Warmup matmuls fill the PE while the first
    stripes stream in so the HAM clock gate is released when real work lands.
    The exp activation table is preloaded via a dummy activation in the same
    dead time.
  - qkT [128, T]: rows 0:64 = q^T, 64:128 = k^T (fused [Wq | Wk] weights,
    xT chunk moving); k^T half shifted to a base-0 tile via SBUF->SBUF DMA on
    the gpsimd software-DGE queue (off the FIFO hardware queues that carry x).
  - v computed directly in NATURAL layout (no transposes): per (t-chunk, c):
    lhsT = xT chunk [128c, 128t] (stationary, FWL), rhs = Wv [128c, 64] moving
    -> psum [128t, 64].
  - attention in S^T layout (keys j on partitions, queries i on free):
    S^T(j-chunk, i-range) = kT_j.T @ qT, 1024-wide column groups. Scale+exp
    fused on ScalarE (PSUM -> SBUF bf16). Causal: only j <= i blocks are
    computed; leading 128-col diagonal block gets an upper-tri mask multiply.
  - AV with probs STATIONARY: out_nat[i-chunk] += prb[:, i-chunk].T @ [v_j|1]
    (65 moving cols per (j,i-chunk) step - half the streaming columns of the
    v-stationary form) accumulating natural-layout [128, 65] psum regions.
    Row... col 64 accumulates the softmax denominators for free. Pair (g,jj)
    completes output chunk i=jj (its diagonal), so finalize (reciprocal of
    col 64 + tensor_scalar multiply -> bf16) streams through the whole
    attention phase and the tail after the last exp is tiny. One merged
    output DMA per 4 chunks.
  - QKV projection stripes 2,3 interleave into the group-0 attention pipeline
    so the PE, ScalarE (exp), and DVE all stay busy concurrently.

No max-subtraction in softmax: scores * C**-0.5 are bounded (|s| < ~3), exp is
safe in fp32, and the result is mathematically identical to jax.nn.softmax.
Output is bf16 on-device (rel err ~6e-3 total), upcast to f32 on the host.
"""

import ml_dtypes
import numpy as np

import concourse.bass as bass
import concourse.tile as tile
from concourse import bacc, mybir
from concourse.bass import ds, ts
from concourse.masks import make_upper_triangular

B, T, C, H = 8, 2048, 768, 64
P = 128
NCH = C // P          # 6 contraction chunks for QKV
GW = 1024             # attention output column-group width
NG = T // GW          # 2 groups
NT = T // P           # 16 t-chunks
JPG = GW // P         # 8 j-chunks per group
SCALE = float(C) ** -0.5

F32 = mybir.dt.float32
BF16 = mybir.dt.bfloat16
EXP = mybir.ActivationFunctionType.Exp


def _emit(tc: tile.TileContext, ctx, xT, wqk, wv, out):
    nc = tc.nc

    consts = ctx.enter_context(tc.tile_pool(name="consts", bufs=1))
    xpool = ctx.enter_context(tc.tile_pool(name="x", bufs=1))
    qpool = ctx.enter_context(tc.tile_pool(name="qkv", bufs=1))

    tri = consts.tile([P, P], BF16)
    make_upper_triangular(nc, tri[:], val=1.0, diag=True)
    scratch = consts.tile([1, 1], F32)

    # weights split across the two HWDGE queues to balance the head load
    w_qk = consts.tile([P, NCH, P], BF16)
    nc.scalar.dma_start(w_qk[:], wqk.rearrange("(o p) m -> p o m", p=P))
    w_v = consts.tile([P, NCH, H], BF16)
    nc.sync.dma_start(w_v[:], wv.rearrange("(o p) m -> p o m", p=P))

    # xT in four 512-col stripes, each split across the two HWDGE queues
    xT_sb = xpool.tile([P, NCH, T], BF16)
    xTr = xT.rearrange("(o p) t -> p o t", p=P)
    for s in range(4):
        nc.scalar.dma_start(xT_sb[:, 0:3, ts(s, 512)], xTr[:, 0:3, ts(s, 512)])
        nc.sync.dma_start(xT_sb[:, 3:6, ts(s, 512)], xTr[:, 3:6, ts(s, 512)])
        if s == 1:
            # preload the exp activation table during the DMA dead time so the
            # first real exp doesn't pay the ~2.7us table-load cost
            nc.scalar.activation(scratch[:], scratch[:], EXP)

    qkT = qpool.tile([P, T], BF16)
    kT = qpool.tile([H, T], BF16)
    v_sb = qpool.tile([P, NT, H + 1], BF16)
    nc.vector.memset(v_sb[:, :, H : H + 1], 1.0)

    # warmup: dummy matmuls fill the PE while the first x stripes stream in,
    # so the HAM clock gate is already released when real work arrives
    dum = qpool.tile([P, 512], BF16)
    nc.vector.memset(dum[:], 0.0)
    with tc.tile_pool(name="warm", bufs=2, space="PSUM") as wp:
        for w in range(12):
            dps = wp.tile([P, 512], F32, tag="w", name=f"warm_{w}")
            nc.tensor.matmul(dps[:], dum[:, 0:P], dum[:], start=True, stop=True)

    # shared PSUM pools: sp serves both projection stripes and S^T chunks
    sp = ctx.enter_context(tc.tile_pool(name="spsum", bufs=3, space="PSUM"))
    op = ctx.enter_context(tc.tile_pool(name="opsum", bufs=2, space="PSUM"))
    pb = ctx.enter_context(tc.tile_pool(name="probs", bufs=6))
    fin = ctx.enter_context(tc.tile_pool(name="fin", bufs=3))

    def emit_proj_qk(s):
        # one 512-col stripe of q|k transposed (fused [Wq | Wk] weights)
        ps = sp.tile([P, GW], F32, tag="s", name=f"projqk_{s}")
        for c in range(NCH):
            nc.tensor.matmul(
                ps[:, 0:512],
                w_qk[:, c, :],
                xT_sb[:, c, ts(s, 512)],
                start=(c == 0),
                stop=(c == NCH - 1),
            )
        nc.vector.tensor_copy(qkT[:, ts(s, 512)], ps[:, 0:512])
        # k^T shift to base partition 0 on the gpsimd software-DGE queue:
        # keeps it off the FIFO hardware queues that carry the bulk x stream
        nc.gpsimd.dma_start(kT[:, ts(s, 512)], qkT[H:P, ts(s, 512)])

    def emit_proj_v(s):
        # v in natural layout: xT chunk stationary (FWL), Wv moving
        ps = sp.tile([P, GW], F32, tag="s", name=f"projv_{s}")
        for i in range(4):
            t = 4 * s + i
            reg = ps[:, ds(H * i, H)]
            for c in range(NCH):
                nc.tensor.matmul(
                    reg,
                    xT_sb[:, c, ts(t, P)],
                    w_v[:, c, :],
                    start=(c == 0),
                    stop=(c == NCH - 1),
                )
            nc.vector.tensor_copy(v_sb[:, t, 0:H], reg)

    def emit_probs(g, jj):
        istart = max(g * GW, jj * P)
        n = (g + 1) * GW - istart
        sps = sp.tile([P, GW], F32, tag="s")
        for h in range(0, n, 512):
            nh = min(512, n - h)
            nc.tensor.matmul(
                sps[:, h : h + nh],
                kT[:, ts(jj, P)],
                qkT[0:H, ds(istart + h, nh)],
                start=True,
                stop=True,
            )
        prb = pb.tile([P, GW], BF16, tag="p")
        nc.scalar.activation(prb[:, :n], sps[:, :n], EXP, scale=SCALE)
        if jj >= JPG * g:
            # leading 128 cols are the diagonal block: upper-tri (j<=i) mask
            nc.vector.tensor_mul(out=prb[:, :P], in0=prb[:, :P], in1=tri[:])
        return prb

    # QK stripes 0-1 first: the S^T chain (and thus ScalarE's exp stream)
    # depends only on these, not on the v projections.
    emit_proj_qk(0)
    emit_proj_qk(1)

    # remaining projection units interleave into group-0 attention; they must
    # be emitted on the PE queue before anything that consumes their outputs,
    # but late enough that their input data has landed by the time the PE
    # reaches them (avoids head-of-line blocking + a HAM warmup reset)
    inject = {
        0: [("v", 1)],
        3: [("qk", 2)],
        4: [("v", 2)],
        5: [("qk", 3)],
        6: [("v", 3)],
    }

    ops_by_g = {}
    onat_by_q = {}
    LOOKAHEAD = 2
    prb_queue = []
    idx = 0
    for g in range(NG):
        npair = JPG * g + JPG
        prb_queue += [emit_probs(g, jj) for jj in range(LOOKAHEAD)]
        if g == 0:
            emit_proj_v(0)
        for jj in range(npair):
            prb = prb_queue.pop(0)
            if jj + LOOKAHEAD < npair:
                prb_queue.append(emit_probs(g, jj + LOOKAHEAD))

            if jj == 0:
                # two half-group tiles: a [128, 8, 65] f32 region would
                # straddle a 2KB PSUM bank boundary, which a matmul
                # accumulation region must not cross
                ops_by_g[g] = [
                    op.tile([P, 4, H + 1], F32, tag="o", name=f"ops_{g}_{hh}")
                    for hh in range(2)
                ]
            istart = max(g * GW, jj * P)
            # AV with probs stationary: one 65-col matmul per output i-chunk,
            # accumulating natural-layout [128, 65] psum regions
            # start=True clears the has_written bits of the WHOLE bank, so only
            # the first matmul into each bank may set it; the other regions
            # self-initialize via flags=0 overwrite-where-bit-unset semantics
            for ii in range(max(jj, JPG * g), JPG * g + JPG):
                il = ii - JPG * g
                nc.tensor.matmul(
                    ops_by_g[g][il // 4][:, il % 4, :],
                    prb[:, ds(ii * P - istart, P)],
                    v_sb[:, jj, :],
                    start=(jj == 0 and il % 4 == 0),
                    stop=(jj == ii),
                    skip_group_check=True,
                )

            for kind, s in inject.get(idx, ()):
                if kind == "qk":
                    emit_proj_qk(s)
                else:
                    emit_proj_v(s)

            # pair (g, jj) completes output chunk i=jj (its diagonal block):
            # normalize it now so finalize streams through the whole phase
            if jj >= JPG * g:
                quad, slot = jj // 4, jj % 4
                if slot == 0:
                    onat_by_q[quad] = fin.tile(
                        [P, 4, H], BF16, tag="onat", name=f"onat_{quad}"
                    )
                o_nat = onat_by_q[quad]
                il = jj - JPG * g
                reg = ops_by_g[g][il // 4][:, il % 4, :]
                rch = fin.tile([P, 1], F32, tag="rch")
                nc.vector.reciprocal(rch[:], reg[:, H : H + 1])
                nc.vector.tensor_scalar_mul(o_nat[:, slot, :], reg[:, 0:H], rch[:])
                if slot == 3:
                    ov = out.rearrange("(a p) h -> p a h", p=P)
                    nc.sync.dma_start(ov[:, ds(quad * 4, 4), :], o_nat[:])
            idx += 1


def build():
    from contextlib import ExitStack

    nc = bacc.Bacc("TRN2", target_bir_lowering=False, debug=False, num_devices=B)
    xT = nc.dram_tensor("xT", [C, T], BF16, kind="ExternalInput").ap()
    wqk = nc.dram_tensor("wqk", [C, P], BF16, kind="ExternalInput").ap()
    wv = nc.dram_tensor("wv", [C, H], BF16, kind="ExternalInput").ap()
    out = nc.dram_tensor("o", [T, H], BF16, kind="ExternalOutput").ap()
    with tile.TileContext(nc) as tc, ExitStack() as ctx:
        _emit(tc, ctx, xT, wqk, wv, out)
    nc.compile()
    return nc


_NC = None


def _get_nc():
    global _NC
    if _NC is None:
        _NC = build()
    return _NC


def make_in_maps(x, Wk, Wq, Wv):
    bf = ml_dtypes.bfloat16
    wqk = np.ascontiguousarray(np.concatenate([Wq, Wk], axis=1)).astype(bf)
    wv = np.ascontiguousarray(np.asarray(Wv)).astype(bf)
    return [
        {
            "xT": np.ascontiguousarray(np.asarray(x[b]).T).astype(bf),
            "wqk": wqk,
            "wv": wv,
        }
        for b in range(B)
    ]


def kernel(x, Wk, Wq, Wv):
    from concourse.bass_utils import run_bass_kernel_spmd

    nc = _get_nc()
    in_maps = make_in_maps(x, Wk, Wq, Wv)
    r = run_bass_kernel_spmd(nc, in_maps, core_ids=list(range(B)))
    out = np.stack([np.asarray(r.results[b]["o"]) for b in range(B)])
    return np.ascontiguousarray(out.astype(np.float32))
